# revision 2
# baseline (speedup 1.0000x reference)
"""Trainium2 Bass kernel for nn_Attention_4183298146960.

GQA causal attention layer: B=2, S=2048, HIDDEN=2048, 16 q heads / 4 kv heads,
head_dim=128, RoPE (interleaved pairs), causal softmax, output projection.

Sharding (8 cores, SPMD-uniform program):
  core c owns q heads {2c, 2c+1} and kv head c//2, for BOTH batches.

Pipeline (v2): fully interleaved per 512-token tile. For t-tile j
(b=j//4, qt=j%4): QKV projection (q0,q1,k,v as four N=512 column blocks of
one packed weight), RoPE on q/k, PE-transpose of v to token-major, then
immediately the causal attention chunk (b,qt) over k-tiles 0..4qt+3, its
AllGather, and (deferred by 2 chunks) the W_o matmul for an earlier chunk.
This keeps ScalarE (exp) and DVE (mask/denominator) busy under the PE-bound
QKV/W_o matmul stream instead of serializing phases.

Layouts on device (partition dim first):
  feature-major qT/kT [head_dim, tokens] for scores; token-major v
  [tokens, head_dim] for PV; scores computed transposed [k, q], both heads
  side by side in one [128, 1024] PSUM tile so softmax needs one exp /
  mask-mult / denominator-add per k-tile. No max-subtraction (scores are
  O(+-10), exp is fp32-safe); denominator via ones-matmul partition-reduce;
  probabilities stay unnormalized until after PV.
  RoPE head dims are permuted [even | odd] via host-side W row permutation
  so the rotation is a 64-partition swap (SBUF->SBUF DMA) + DVE ops.
"""

import numpy as np
import ml_dtypes

import concourse.bass as bass
import concourse.mybir as mybir
import concourse.tile as tile
from concourse import bacc
from concourse.bass_utils import run_bass_kernel_spmd

BF16 = ml_dtypes.bfloat16

HEADS = 16
KV_HEADS = 4
HIDDEN = 2048
HD = 128
S = 2048
B = 2
T = B * S                      # 4096 token axis (both batches)
HT = HIDDEN // 128             # 16 hidden tiles
SCALE = 1.0 / float(np.sqrt(HD))
RG8 = [[0, 1, 2, 3, 4, 5, 6, 7]]

_COMPILED = None


def _build():
    dt = mybir.dt
    nc = bacc.Bacc("TRN2", target_bir_lowering=False, debug=False, num_devices=8)

    xT = nc.dram_tensor("xT", [128, HT, T], dt.bfloat16, kind="ExternalInput")
    wqkv = nc.dram_tensor("wqkv", [128, HT, 512], dt.bfloat16, kind="ExternalInput")
    wo = nc.dram_tensor("wo", [128, HT, 256], dt.bfloat16, kind="ExternalInput")
    cc = nc.dram_tensor("cc", [128, T], dt.bfloat16, kind="ExternalInput")
    ss = nc.dram_tensor("ss", [128, T], dt.bfloat16, kind="ExternalInput")
    msk = nc.dram_tensor("msk", [128, 4, 1024], dt.bfloat16, kind="ExternalInput")
    ones128 = nc.dram_tensor("ones128", [128, 128], dt.bfloat16, kind="ExternalInput")
    ident = nc.dram_tensor("ident", [128, 128], dt.bfloat16, kind="ExternalInput")
    outT = nc.dram_tensor("outT", [256, T], dt.float32, kind="ExternalOutput")

    mult = mybir.AluOpType.mult
    add = mybir.AluOpType.add
    Exp = mybir.ActivationFunctionType.Exp

    with tile.TileContext(nc) as tc:
        with (
            tc.tile_pool(name="const", bufs=1) as constp,
            tc.tile_pool(name="dram", bufs=1, space="DRAM") as dram,
            tc.tile_pool(name="xp", bufs=2) as xp,
            tc.tile_pool(name="rp", bufs=3) as rp,
            tc.tile_pool(name="probs", bufs=4) as probs,
            tc.tile_pool(name="smallp", bufs=2) as smallp,
            tc.tile_pool(name="ap", bufs=3) as apool,
            tc.tile_pool(name="wosb", bufs=2) as wosb,
            tc.tile_pool(name="outp", bufs=2) as outp,
            # PSUM: qk 2 banks + scores 2x2 banks + pv 2 banks = 8
            tc.tile_pool(name="qkps", bufs=2, space="PSUM") as qkps,
            tc.tile_pool(name="spool", bufs=2, space="PSUM") as spool,
            tc.tile_pool(name="pvp", bufs=1, space="PSUM") as pvp,
        ):
            qcat = constp.tile([128, 2 * T], dt.bfloat16)   # 2 local q heads
            kT = constp.tile([128, T], dt.bfloat16)
            vsb = constp.tile([128, T], dt.bfloat16)        # token-major v
            wqkv_sb = constp.tile([128, HT, 512], dt.bfloat16)
            wo_sb = constp.tile([128, HT, 256], dt.bfloat16)
            cc_sb = constp.tile([128, T], dt.bfloat16)
            ss_sb = constp.tile([128, T], dt.bfloat16)
            msk_sb = constp.tile([128, 4, 1024], dt.bfloat16)
            ones_sb = constp.tile([128, 128], dt.bfloat16)
            id_sb = constp.tile([128, 128], dt.bfloat16)

            # weight quarters first so first MMs can start early
            for hq in range(4):
                nc.sync.dma_start(
                    wqkv_sb[:, hq * 4:(hq + 1) * 4, :], wqkv[:, hq * 4:(hq + 1) * 4, :]
                )

            def load_x(j, x_sb):
                for hq in range(4):
                    nc.sync.dma_start(
                        x_sb[:, hq * 4:(hq + 1) * 4, :],
                        xT[:, hq * 4:(hq + 1) * 4, j * 512:(j + 1) * 512],
                    )

            x_tiles = {}
            x_tiles[0] = xp.tile([128, HT, 512], dt.bfloat16, name="x0", tag="x")
            load_x(0, x_tiles[0])

            # remaining constants after the first x tile is on its way
            nc.sync.dma_start(cc_sb[:], cc[:])
            nc.sync.dma_start(ss_sb[:], ss[:])
            nc.sync.dma_start(msk_sb[:], msk[:])
            nc.sync.dma_start(ones_sb[:], ones128[:])
            nc.sync.dma_start(id_sb[:], ident[:])
            nc.sync.dma_start(wo_sb[:], wo[:])

            def emit_qkv(j, x_sb):
                """QKV + RoPE + v-transpose for t-tile j (512 tokens)."""
                tsl = bass.ts(j, 512)
                for ft in range(4):
                    ps = qkps.tile([128, 512], dt.float32, tag="qk")
                    for ht in range(HT):
                        nc.tensor.matmul(
                            ps[:],
                            lhsT=wqkv_sb[:, ht, ft * 128:(ft + 1) * 128],
                            rhs=x_sb[:, ht, :],
                            start=(ht == 0),
                            stop=(ht == HT - 1),
                        )
                    if ft < 3:
                        # q0, q1, k: RoPE
                        sbq = rp.tile([128, 512], dt.bfloat16, name="sbq")
                        nc.scalar.copy(sbq[:], ps[:])
                        tmp = rp.tile([128, 512], dt.bfloat16, name="tmp")
                        nc.gpsimd.dma_start(tmp[0:64, :], sbq[64:128, :])
                        nc.gpsimd.dma_start(tmp[64:128, :], sbq[0:64, :])
                        qcc = rp.tile([128, 512], dt.bfloat16, name="qcc")
                        nc.vector.tensor_tensor(qcc[:], sbq[:], cc_sb[:, tsl], mult)
                        qss = rp.tile([128, 512], dt.bfloat16, name="qss")
                        nc.vector.tensor_tensor(qss[:], tmp[:], ss_sb[:, tsl], mult)
                        if ft < 2:
                            dst = qcat[:, ft * T + j * 512: ft * T + (j + 1) * 512]
                        else:
                            dst = kT[:, tsl]
                        nc.vector.tensor_tensor(dst, qcc[:], qss[:], add)
                    else:
                        # v: feature-major -> PE transpose -> token-major
                        vf = rp.tile([128, 512], dt.bfloat16, name="vf")
                        nc.scalar.copy(vf[:], ps[:])
                        pt = qkps.tile(
                            [128, 512], dt.bfloat16, tag="qk", name="pt",
                            padded_shape=[128, 1024],
                        )
                        for st in range(4):
                            nc.tensor.transpose(
                                pt[:, st * 128:(st + 1) * 128],
                                vf[:, st * 128:(st + 1) * 128],
                                id_sb[:],
                            )
                        nc.vector.tensor_copy(vsb[:, tsl], pt[:])

            def emit_attn(b, qt):
                """Causal attention chunk for 512 q tokens; both local heads."""
                kts = 4 * qt + 4
                pv = pvp.tile([128, 1024], dt.float32, tag="pv")
                acc = smallp.tile([128, 1024], dt.bfloat16, name="acc", tag="acc")
                for kt in range(kts):
                    r = kt - 4 * qt
                    ps_s = spool.tile([128, 1024], dt.float32, tag="sc")
                    for hl in range(2):
                        nc.tensor.matmul(
                            ps_s[:, hl * 512:(hl + 1) * 512],
                            lhsT=kT[:, b * S + kt * 128: b * S + (kt + 1) * 128],
                            rhs=qcat[:, hl * T + b * S + qt * 512:
                                     hl * T + b * S + (qt + 1) * 512],
                            start=True,
                            stop=True,
                        )
                    prob = probs.tile([128, 1024], dt.bfloat16, tag="prob")
                    if r >= 0:
                        stg = probs.tile([128, 1024], dt.bfloat16, name="stg", tag="stg")
                        nc.scalar.activation(stg[:], ps_s[:], Exp, scale=SCALE)
                        nc.vector.tensor_tensor(prob[:], stg[:], msk_sb[:, r, :], mult)
                    else:
                        nc.scalar.activation(prob[:], ps_s[:], Exp, scale=SCALE)
                    for hl in range(2):
                        nc.tensor.matmul(
                            pv[:, hl * 512:(hl + 1) * 512],
                            lhsT=vsb[:, (b * HT + kt) * 128: (b * HT + kt + 1) * 128],
                            rhs=prob[:, hl * 512:(hl + 1) * 512],
                            start=(kt == 0), stop=(kt == kts - 1),
                        )
                    if kt == 0:
                        nc.vector.tensor_copy(acc[:], prob[:])
                    else:
                        nc.vector.tensor_add(acc[:], acc[:], prob[:])
                # partition-reduce + broadcast denominators in one matmul per head
                ps_den = spool.tile([128, 1024], dt.float32, tag="sc", name="ps_den")
                for hl in range(2):
                    nc.tensor.matmul(
                        ps_den[:, hl * 512:(hl + 1) * 512],
                        lhsT=ones_sb[:], rhs=acc[:, hl * 512:(hl + 1) * 512],
                        start=True, stop=True,
                    )
                den_sb = smallp.tile([128, 1024], dt.float32, name="den", tag="den")
                nc.vector.reciprocal_approx_fast(den_sb[:], ps_den[:])
                attn_chunk = dram.tile(
                    [256, 512], dt.bfloat16, name=f"attnc{b}_{qt}", tag=f"ac{b}{qt}"
                )
                for hl in range(2):
                    attn_sb = apool.tile([128, 512], dt.bfloat16, tag="asb")
                    nc.vector.tensor_tensor(
                        attn_sb[:], pv[:, hl * 512:(hl + 1) * 512],
                        den_sb[:, hl * 512:(hl + 1) * 512], mult,
                    )
                    nc.sync.dma_start(attn_chunk[hl * 128:(hl + 1) * 128, :], attn_sb[:])
                ag_out = dram.tile(
                    [HT, 128, 512], dt.bfloat16,
                    addr_space="Shared", name=f"agout{b}_{qt}", tag=f"ag{b}{qt}",
                )
                nc.gpsimd.collective_compute(
                    "AllGather", mybir.AluOpType.bypass, replica_groups=RG8,
                    ins=[attn_chunk.opt()], outs=[ag_out.opt()],
                )
                return ag_out

            def emit_wo(b, qt, ag_out):
                """W_o for one chunk's 512 tokens; deferred so the PE stream
                never waits on a fresh gather."""
                asb = wosb.tile([128, HT, 512], dt.bfloat16, tag="asb")
                for g in range(4):
                    nc.sync.dma_start(
                        asb[:, g * 4:(g + 1) * 4, :], ag_out[g * 4:(g + 1) * 4, :, :]
                    )
                for ct in range(2):
                    ps_o = qkps.tile([128, 512], dt.float32, tag="qk", name="ps_o")
                    for dtt in range(HT):
                        nc.tensor.matmul(
                            ps_o[:],
                            lhsT=wo_sb[:, dtt, ct * 128:(ct + 1) * 128],
                            rhs=asb[:, dtt, :],
                            start=(dtt == 0), stop=(dtt == HT - 1),
                        )
                    o_sb = outp.tile([128, 512], dt.float32, tag="osb")
                    nc.scalar.copy(o_sb[:], ps_o[:])
                    nc.sync.dma_start(
                        outT[ct * 128:(ct + 1) * 128, b * S + qt * 512: b * S + (qt + 1) * 512],
                        o_sb[:],
                    )

            pending_wo = []
            for j in range(8):
                b, qt = j // 4, j % 4
                if j + 1 < 8:
                    x_tiles[j + 1] = xp.tile(
                        [128, HT, 512], dt.bfloat16, name=f"x{j + 1}", tag="x"
                    )
                    load_x(j + 1, x_tiles[j + 1])
                emit_qkv(j, x_tiles[j])
                ag = emit_attn(b, qt)
                pending_wo.append((b, qt, ag))
                if len(pending_wo) > 2:
                    emit_wo(*pending_wo.pop(0))
            for w in pending_wo:
                emit_wo(*w)
    nc.compile()
    return nc


# host-side input prep ------------------------------------------------------

_PERM = np.concatenate([np.arange(0, HD, 2), np.arange(1, HD, 2)])


def _rope_tables():
    freq = 1.0 / (10000.0 ** (np.arange(0, HD, 2, dtype=np.float64) / HD))
    pos = np.arange(S, dtype=np.float64)
    ang = np.outer(pos, freq)                       # [S, 64]
    cos = np.cos(ang).T.astype(np.float32)          # [64, S]
    sin = np.sin(ang).T.astype(np.float32)
    cc1 = np.concatenate([cos, cos], 0)             # [128, S]
    ss1 = np.concatenate([-sin, sin], 0)            # [128, S]
    return (np.tile(cc1, (1, B)).astype(BF16), np.tile(ss1, (1, B)).astype(BF16))


def _prep_inputs(x, W_qkv, W_o):
    x = np.asarray(x, dtype=np.float32)
    W_qkv = np.asarray(W_qkv, dtype=np.float32)
    W_o = np.asarray(W_o, dtype=np.float32)

    xx = np.concatenate([x[0], x[1]], axis=0)       # [4096, 2048]
    xTd = np.ascontiguousarray(
        xx.T.reshape(HT, 128, T).transpose(1, 0, 2)
    ).astype(BF16)                                   # [128, HT, 4096]

    cc, ss = _rope_tables()

    mask = np.zeros((128, 4, 1024), dtype=np.float32)
    ii = np.arange(128)[:, None]
    jj = np.arange(512)[None, :]
    for r in range(4):
        m = (jj >= ii + 128 * r)
        mask[:, r, 0:512] = m
        mask[:, r, 512:1024] = m
    mask = mask.astype(BF16)

    ones128 = np.ones((128, 128), dtype=np.float32).astype(BF16)
    ident = np.eye(128, dtype=np.float32).astype(BF16)

    in_maps = []
    for c in range(8):
        kh = c // 2
        qr = W_qkv[256 * c: 256 * (c + 1)]           # rows of q heads 2c,2c+1
        qr = qr.reshape(2, HD, HIDDEN)[:, _PERM, :].reshape(256, HIDDEN)
        kr = W_qkv[HIDDEN + 128 * kh: HIDDEN + 128 * (kh + 1)][_PERM, :]
        vr = W_qkv[HIDDEN + 512 + 128 * kh: HIDDEN + 512 + 128 * (kh + 1)]
        wqkvT = np.ascontiguousarray(
            np.concatenate([qr, kr, vr], 0).T.reshape(HT, 128, 512).transpose(1, 0, 2)
        ).astype(BF16)                               # [128, HT, 512]
        woT = np.ascontiguousarray(
            W_o[256 * c: 256 * (c + 1)].T.reshape(HT, 128, 256).transpose(1, 0, 2)
        ).astype(BF16)
        in_maps.append({
            "xT": xTd, "wqkv": wqkvT, "wo": woT,
            "cc": cc, "ss": ss, "msk": mask, "ones128": ones128, "ident": ident,
        })
    return in_maps


def kernel(x, W_qkv, W_o):
    global _COMPILED
    if _COMPILED is None:
        _COMPILED = _build()
    nc = _COMPILED
    in_maps = _prep_inputs(x, W_qkv, W_o)
    res = run_bass_kernel_spmd(nc, in_maps, list(range(8)))
    out = np.empty((B, S, HIDDEN), dtype=np.float32)
    for c in range(8):
        oT = res.results[c]["outT"]                  # [256, 4096]
        out[:, :, 256 * c: 256 * (c + 1)] = oT.reshape(256, B, S).transpose(1, 2, 0)
    return out


# revision 6
# speedup vs baseline: 1.0670x; 1.0670x over previous
"""Trainium2 Bass kernel for nn_Attention_4183298146960.

GQA causal attention layer: B=2, S=2048, HIDDEN=2048, 16 q heads / 4 kv heads,
head_dim=128, RoPE (interleaved pairs), causal softmax, output projection.

Sharding (8 cores, SPMD-uniform program):
  core c owns q heads {2c, 2c+1} and kv head c//2, for BOTH batches.

Pipeline (v2): fully interleaved per 512-token tile. For t-tile j
(b=j//4, qt=j%4): QKV projection (q0,q1,k,v as four N=512 column blocks of
one packed weight), RoPE on q/k, PE-transpose of v to token-major, then
immediately the causal attention chunk (b,qt) over k-tiles 0..4qt+3, its
AllGather, and (deferred by 2 chunks) the W_o matmul for an earlier chunk.
This keeps ScalarE (exp) and DVE (mask/denominator) busy under the PE-bound
QKV/W_o matmul stream instead of serializing phases.

Layouts on device (partition dim first):
  feature-major qT/kT [head_dim, tokens] for scores; token-major v
  [tokens, head_dim] for PV; scores computed transposed [k, q], both heads
  side by side in one [128, 1024] PSUM tile so softmax needs one exp /
  mask-mult / denominator-add per k-tile. No max-subtraction (scores are
  O(+-10), exp is fp32-safe); denominator via ones-matmul partition-reduce;
  probabilities stay unnormalized until after PV.
  RoPE head dims are permuted [even | odd] via host-side W row permutation
  so the rotation is a 64-partition swap (SBUF->SBUF DMA) + DVE ops.
"""

import numpy as np
import ml_dtypes

import concourse.bass as bass
import concourse.mybir as mybir
import concourse.tile as tile
from concourse import bacc
from concourse.bass_utils import run_bass_kernel_spmd

BF16 = ml_dtypes.bfloat16

HEADS = 16
KV_HEADS = 4
HIDDEN = 2048
HD = 128
S = 2048
B = 2
T = B * S                      # 4096 token axis (both batches)
HT = HIDDEN // 128             # 16 hidden tiles
SCALE = 1.0 / float(np.sqrt(HD))
RG8 = [[0, 1, 2, 3, 4, 5, 6, 7]]

_COMPILED = None


def _build():
    dt = mybir.dt
    nc = bacc.Bacc("TRN2", target_bir_lowering=False, debug=False, num_devices=8)

    xT = nc.dram_tensor("xT", [128, HT, T], dt.bfloat16, kind="ExternalInput")
    wqkv = nc.dram_tensor("wqkv", [128, HT, 512], dt.bfloat16, kind="ExternalInput")
    wo = nc.dram_tensor("wo", [128, HT, 256], dt.bfloat16, kind="ExternalInput")
    cc = nc.dram_tensor("cc", [128, T], dt.bfloat16, kind="ExternalInput")
    ss = nc.dram_tensor("ss", [128, T], dt.bfloat16, kind="ExternalInput")
    msk = nc.dram_tensor("msk", [128, 4, 1024], dt.bfloat16, kind="ExternalInput")
    ones128 = nc.dram_tensor("ones128", [128, 128], dt.bfloat16, kind="ExternalInput")
    ident = nc.dram_tensor("ident", [128, 128], dt.bfloat16, kind="ExternalInput")
    outT = nc.dram_tensor("outT", [256, T], dt.float32, kind="ExternalOutput")

    mult = mybir.AluOpType.mult
    add = mybir.AluOpType.add
    Exp = mybir.ActivationFunctionType.Exp

    with tile.TileContext(nc) as tc:
        with (
            tc.tile_pool(name="const", bufs=1) as constp,
            tc.tile_pool(name="dram", bufs=1, space="DRAM") as dram,
            tc.tile_pool(name="xp", bufs=2) as xp,
            tc.tile_pool(name="rp", bufs=3) as rp,
            tc.tile_pool(name="probs", bufs=4) as probs,
            tc.tile_pool(name="smallp", bufs=2) as smallp,
            tc.tile_pool(name="ap", bufs=3) as apool,
            tc.tile_pool(name="wosb", bufs=2) as wosb,
            tc.tile_pool(name="outp", bufs=2) as outp,
            # PSUM: qk 2 banks + scores 2x2 banks + pv 2 banks = 8
            tc.tile_pool(name="qkps", bufs=2, space="PSUM") as qkps,
            tc.tile_pool(name="spool", bufs=2, space="PSUM") as spool,
            tc.tile_pool(name="pvp", bufs=1, space="PSUM") as pvp,
        ):
            qcat = constp.tile([128, 2 * T], dt.bfloat16)   # 2 local q heads
            kT = constp.tile([128, T], dt.bfloat16)
            vsb = constp.tile([128, T], dt.bfloat16)        # token-major v
            wqkv_sb = constp.tile([128, HT, 512], dt.bfloat16)
            wo_sb = constp.tile([128, HT, 256], dt.bfloat16)
            cc_sb = constp.tile([128, T], dt.bfloat16)
            ss_sb = constp.tile([128, T], dt.bfloat16)
            msk_sb = constp.tile([128, 4, 1024], dt.bfloat16)
            ones_sb = constp.tile([128, 128], dt.bfloat16)
            id_sb = constp.tile([128, 128], dt.bfloat16)

            def load_x(j, x_sb):
                for hq in range(4):
                    nc.sync.dma_start(
                        x_sb[:, hq * 4:(hq + 1) * 4, :],
                        xT[:, hq * 4:(hq + 1) * 4, j * 512:(j + 1) * 512],
                    )

            # interleave weight/x quarters so the first ht-tile MMs start ASAP
            x_tiles = {}
            x_tiles[0] = xp.tile([128, HT, 512], dt.bfloat16, name="x0", tag="x")
            for hq in range(4):
                nc.sync.dma_start(
                    wqkv_sb[:, hq * 4:(hq + 1) * 4, :], wqkv[:, hq * 4:(hq + 1) * 4, :]
                )
                nc.sync.dma_start(
                    x_tiles[0][:, hq * 4:(hq + 1) * 4, :],
                    xT[:, hq * 4:(hq + 1) * 4, 0:512],
                )

            # remaining constants after the first x tile is on its way; rope
            # tables first (needed by tile 0's rope), output-side consts last
            nc.sync.dma_start(cc_sb[:, 0:512], cc[:, 0:512])
            nc.sync.dma_start(ss_sb[:, 0:512], ss[:, 0:512])
            nc.sync.dma_start(id_sb[:], ident[:])
            nc.sync.dma_start(cc_sb[:, 512:T], cc[:, 512:T])
            nc.sync.dma_start(ss_sb[:, 512:T], ss[:, 512:T])
            nc.sync.dma_start(msk_sb[:], msk[:])
            nc.sync.dma_start(ones_sb[:], ones128[:])
            nc.sync.dma_start(wo_sb[:], wo[:])

            def emit_qkv(j, x_sb):
                """QKV + RoPE + v-transpose for t-tile j (512 tokens)."""
                tsl = bass.ts(j, 512)
                for ft in range(4):
                    ps = qkps.tile([128, 512], dt.float32, tag="qk")
                    for ht in range(HT):
                        nc.tensor.matmul(
                            ps[:],
                            lhsT=wqkv_sb[:, ht, ft * 128:(ft + 1) * 128],
                            rhs=x_sb[:, ht, :],
                            start=(ht == 0),
                            stop=(ht == HT - 1),
                        )
                    if ft < 3:
                        # q0, q1, k: RoPE
                        sbq = rp.tile([128, 512], dt.bfloat16, name="sbq")
                        nc.scalar.copy(sbq[:], ps[:])
                        tmp = rp.tile([128, 512], dt.bfloat16, name="tmp")
                        nc.gpsimd.dma_start(tmp[0:64, :], sbq[64:128, :])
                        nc.gpsimd.dma_start(tmp[64:128, :], sbq[0:64, :])
                        qcc = rp.tile([128, 512], dt.bfloat16, name="qcc")
                        nc.vector.tensor_tensor(qcc[:], sbq[:], cc_sb[:, tsl], mult)
                        qss = rp.tile([128, 512], dt.bfloat16, name="qss")
                        nc.vector.tensor_tensor(qss[:], tmp[:], ss_sb[:, tsl], mult)
                        if ft < 2:
                            dst = qcat[:, ft * T + j * 512: ft * T + (j + 1) * 512]
                        else:
                            dst = kT[:, tsl]
                        nc.vector.tensor_tensor(dst, qcc[:], qss[:], add)
                    else:
                        # v: feature-major -> PE transpose -> token-major
                        vf = rp.tile([128, 512], dt.bfloat16, name="vf")
                        nc.scalar.copy(vf[:], ps[:])
                        pt = qkps.tile(
                            [128, 512], dt.bfloat16, tag="qk", name="pt",
                            padded_shape=[128, 1024],
                        )
                        for st in range(4):
                            nc.tensor.transpose(
                                pt[:, st * 128:(st + 1) * 128],
                                vf[:, st * 128:(st + 1) * 128],
                                id_sb[:],
                            )
                        nc.vector.tensor_copy(vsb[:, tsl], pt[:])

            def emit_attn(b, qt):
                """Causal attention chunk for 512 q tokens; both local heads."""
                kts = 4 * qt + 4
                pv = pvp.tile([128, 1024], dt.float32, tag="pv")
                acc = smallp.tile([128, 1024], dt.bfloat16, name="acc", tag="acc")
                for kt in range(kts):
                    r = kt - 4 * qt
                    ps_s = spool.tile([128, 1024], dt.float32, tag="sc")
                    for hl in range(2):
                        nc.tensor.matmul(
                            ps_s[:, hl * 512:(hl + 1) * 512],
                            lhsT=kT[:, b * S + kt * 128: b * S + (kt + 1) * 128],
                            rhs=qcat[:, hl * T + b * S + qt * 512:
                                     hl * T + b * S + (qt + 1) * 512],
                            start=True,
                            stop=True,
                        )
                    prob = probs.tile([128, 1024], dt.bfloat16, tag="prob")
                    if r >= 0:
                        stg = probs.tile([128, 1024], dt.bfloat16, name="stg", tag="stg")
                        nc.scalar.activation(stg[:], ps_s[:], Exp, scale=SCALE)
                        nc.vector.tensor_tensor(prob[:], stg[:], msk_sb[:, r, :], mult)
                    else:
                        nc.scalar.activation(prob[:], ps_s[:], Exp, scale=SCALE)
                    for hl in range(2):
                        nc.tensor.matmul(
                            pv[:, hl * 512:(hl + 1) * 512],
                            lhsT=vsb[:, (b * HT + kt) * 128: (b * HT + kt + 1) * 128],
                            rhs=prob[:, hl * 512:(hl + 1) * 512],
                            start=(kt == 0), stop=(kt == kts - 1),
                        )
                    if kt == 0:
                        nc.vector.tensor_copy(acc[:], prob[:])
                    else:
                        nc.vector.tensor_add(acc[:], acc[:], prob[:])
                # partition-reduce + broadcast denominators in one matmul per head
                ps_den = spool.tile([128, 1024], dt.float32, tag="sc", name="ps_den")
                for hl in range(2):
                    nc.tensor.matmul(
                        ps_den[:, hl * 512:(hl + 1) * 512],
                        lhsT=ones_sb[:], rhs=acc[:, hl * 512:(hl + 1) * 512],
                        start=True, stop=True,
                    )
                den_sb = smallp.tile([128, 1024], dt.float32, name="den", tag="den")
                nc.vector.reciprocal_approx_fast(den_sb[:], ps_den[:])
                attn_chunk = dram.tile(
                    [256, 512], dt.bfloat16, name=f"attnc{b}_{qt}", tag=f"ac{b}{qt}"
                )
                for hl in range(2):
                    attn_sb = apool.tile([128, 512], dt.bfloat16, tag="asb")
                    nc.vector.tensor_tensor(
                        attn_sb[:], pv[:, hl * 512:(hl + 1) * 512],
                        den_sb[:, hl * 512:(hl + 1) * 512], mult,
                    )
                    nc.sync.dma_start(attn_chunk[hl * 128:(hl + 1) * 128, :], attn_sb[:])
                ag_out = dram.tile(
                    [HT, 128, 512], dt.bfloat16,
                    addr_space="Shared", name=f"agout{b}_{qt}", tag=f"ag{b}{qt}",
                )
                nc.gpsimd.collective_compute(
                    "AllGather", mybir.AluOpType.bypass, replica_groups=RG8,
                    ins=[attn_chunk.opt()], outs=[ag_out.opt()],
                )
                return ag_out

            def emit_wo(b, qt, ag_out):
                """W_o for one chunk's 512 tokens; deferred so the PE stream
                never waits on a fresh gather."""
                asb = wosb.tile([128, HT, 512], dt.bfloat16, tag="asb")
                for g in range(4):
                    nc.sync.dma_start(
                        asb[:, g * 4:(g + 1) * 4, :],
                        ag_out[g * 4:(g + 1) * 4, :, :].transpose([1, 0, 2]),
                    )
                for ct in range(2):
                    ps_o = qkps.tile([128, 512], dt.float32, tag="qk", name="ps_o")
                    for dtt in range(HT):
                        nc.tensor.matmul(
                            ps_o[:],
                            lhsT=wo_sb[:, dtt, ct * 128:(ct + 1) * 128],
                            rhs=asb[:, dtt, :],
                            start=(dtt == 0), stop=(dtt == HT - 1),
                        )
                    o_sb = outp.tile([128, 512], dt.float32, tag="osb")
                    nc.scalar.copy(o_sb[:], ps_o[:])
                    nc.scalar.dma_start(
                        outT[ct * 128:(ct + 1) * 128, b * S + qt * 512: b * S + (qt + 1) * 512],
                        o_sb[:],
                    )

            pending_wo = []
            for j in range(8):
                b, qt = j // 4, j % 4
                if j + 1 < 8:
                    x_tiles[j + 1] = xp.tile(
                        [128, HT, 512], dt.bfloat16, name=f"x{j + 1}", tag="x"
                    )
                    load_x(j + 1, x_tiles[j + 1])
                emit_qkv(j, x_tiles[j])
                ag = emit_attn(b, qt)
                pending_wo.append((b, qt, ag))
                if len(pending_wo) > 3:
                    emit_wo(*pending_wo.pop(0))
            for w in pending_wo:
                emit_wo(*w)
    nc.compile()
    return nc


# host-side input prep ------------------------------------------------------

_PERM = np.concatenate([np.arange(0, HD, 2), np.arange(1, HD, 2)])


def _rope_tables():
    freq = 1.0 / (10000.0 ** (np.arange(0, HD, 2, dtype=np.float64) / HD))
    pos = np.arange(S, dtype=np.float64)
    ang = np.outer(pos, freq)                       # [S, 64]
    cos = np.cos(ang).T.astype(np.float32)          # [64, S]
    sin = np.sin(ang).T.astype(np.float32)
    cc1 = np.concatenate([cos, cos], 0)             # [128, S]
    ss1 = np.concatenate([-sin, sin], 0)            # [128, S]
    return (np.tile(cc1, (1, B)).astype(BF16), np.tile(ss1, (1, B)).astype(BF16))


def _prep_inputs(x, W_qkv, W_o):
    x = np.asarray(x, dtype=np.float32)
    W_qkv = np.asarray(W_qkv, dtype=np.float32)
    W_o = np.asarray(W_o, dtype=np.float32)

    xx = np.concatenate([x[0], x[1]], axis=0)       # [4096, 2048]
    xTd = np.ascontiguousarray(
        xx.T.reshape(HT, 128, T).transpose(1, 0, 2)
    ).astype(BF16)                                   # [128, HT, 4096]

    cc, ss = _rope_tables()

    mask = np.zeros((128, 4, 1024), dtype=np.float32)
    ii = np.arange(128)[:, None]
    jj = np.arange(512)[None, :]
    for r in range(4):
        m = (jj >= ii + 128 * r)
        mask[:, r, 0:512] = m
        mask[:, r, 512:1024] = m
    mask = mask.astype(BF16)

    ones128 = np.ones((128, 128), dtype=np.float32).astype(BF16)
    ident = np.eye(128, dtype=np.float32).astype(BF16)

    in_maps = []
    for c in range(8):
        kh = c // 2
        qr = W_qkv[256 * c: 256 * (c + 1)]           # rows of q heads 2c,2c+1
        qr = qr.reshape(2, HD, HIDDEN)[:, _PERM, :].reshape(256, HIDDEN)
        kr = W_qkv[HIDDEN + 128 * kh: HIDDEN + 128 * (kh + 1)][_PERM, :]
        vr = W_qkv[HIDDEN + 512 + 128 * kh: HIDDEN + 512 + 128 * (kh + 1)]
        wqkvT = np.ascontiguousarray(
            np.concatenate([qr, kr, vr], 0).T.reshape(HT, 128, 512).transpose(1, 0, 2)
        ).astype(BF16)                               # [128, HT, 512]
        woT = np.ascontiguousarray(
            W_o[256 * c: 256 * (c + 1)].T.reshape(HT, 128, 256).transpose(1, 0, 2)
        ).astype(BF16)
        in_maps.append({
            "xT": xTd, "wqkv": wqkvT, "wo": woT,
            "cc": cc, "ss": ss, "msk": mask, "ones128": ones128, "ident": ident,
        })
    return in_maps


def kernel(x, W_qkv, W_o):
    global _COMPILED
    if _COMPILED is None:
        _COMPILED = _build()
    nc = _COMPILED
    in_maps = _prep_inputs(x, W_qkv, W_o)
    res = run_bass_kernel_spmd(nc, in_maps, list(range(8)))
    out = np.empty((B, S, HIDDEN), dtype=np.float32)
    for c in range(8):
        oT = res.results[c]["outT"]                  # [256, 4096]
        out[:, :, 256 * c: 256 * (c + 1)] = oT.reshape(256, B, S).transpose(1, 2, 0)
    return out


# revision 8
# speedup vs baseline: 1.0694x; 1.0022x over previous
"""Trainium2 Bass kernel for nn_Attention_4183298146960.

GQA causal attention layer: B=2, S=2048, HIDDEN=2048, 16 q heads / 4 kv heads,
head_dim=128, RoPE (interleaved pairs), causal softmax, output projection.

Sharding (8 cores, SPMD-uniform program):
  core c owns q heads {2c, 2c+1} and kv head c//2, for BOTH batches.

Pipeline (v2): fully interleaved per 512-token tile. For t-tile j
(b=j//4, qt=j%4): QKV projection (q0,q1,k,v as four N=512 column blocks of
one packed weight), RoPE on q/k, PE-transpose of v to token-major, then
immediately the causal attention chunk (b,qt) over k-tiles 0..4qt+3, its
AllGather, and (deferred by 2 chunks) the W_o matmul for an earlier chunk.
This keeps ScalarE (exp) and DVE (mask/denominator) busy under the PE-bound
QKV/W_o matmul stream instead of serializing phases.

Layouts on device (partition dim first):
  feature-major qT/kT [head_dim, tokens] for scores; token-major v
  [tokens, head_dim] for PV; scores computed transposed [k, q], both heads
  side by side in one [128, 1024] PSUM tile so softmax needs one exp /
  mask-mult / denominator-add per k-tile. No max-subtraction (scores are
  O(+-10), exp is fp32-safe); denominator via ones-matmul partition-reduce;
  probabilities stay unnormalized until after PV.
  RoPE head dims are permuted [even | odd] via host-side W row permutation
  so the rotation is a 64-partition swap (SBUF->SBUF DMA) + DVE ops.
"""

import numpy as np
import ml_dtypes

import concourse.bass as bass
import concourse.mybir as mybir
import concourse.tile as tile
from concourse import bacc
from concourse.bass_utils import run_bass_kernel_spmd

BF16 = ml_dtypes.bfloat16

HEADS = 16
KV_HEADS = 4
HIDDEN = 2048
HD = 128
S = 2048
B = 2
T = B * S                      # 4096 token axis (both batches)
HT = HIDDEN // 128             # 16 hidden tiles
SCALE = 1.0 / float(np.sqrt(HD))
RG8 = [[0, 1, 2, 3, 4, 5, 6, 7]]

_COMPILED = None


def _build():
    dt = mybir.dt
    nc = bacc.Bacc("TRN2", target_bir_lowering=False, debug=False, num_devices=8)

    xT = nc.dram_tensor("xT", [128, HT, T], dt.bfloat16, kind="ExternalInput")
    wqkv = nc.dram_tensor("wqkv", [128, HT, 512], dt.bfloat16, kind="ExternalInput")
    wo = nc.dram_tensor("wo", [128, HT, 256], dt.bfloat16, kind="ExternalInput")
    cc = nc.dram_tensor("cc", [128, T], dt.bfloat16, kind="ExternalInput")
    ss = nc.dram_tensor("ss", [128, T], dt.bfloat16, kind="ExternalInput")
    msk = nc.dram_tensor("msk", [128, 4, 1024], dt.bfloat16, kind="ExternalInput")
    ones128 = nc.dram_tensor("ones128", [128, 128], dt.bfloat16, kind="ExternalInput")
    ident = nc.dram_tensor("ident", [128, 128], dt.bfloat16, kind="ExternalInput")
    outT = nc.dram_tensor("outT", [256, T], dt.float32, kind="ExternalOutput")

    mult = mybir.AluOpType.mult
    add = mybir.AluOpType.add
    Exp = mybir.ActivationFunctionType.Exp

    with tile.TileContext(nc) as tc:
        with (
            tc.tile_pool(name="const", bufs=1) as constp,
            tc.tile_pool(name="dram", bufs=1, space="DRAM") as dram,
            tc.tile_pool(name="xp", bufs=2) as xp,
            tc.tile_pool(name="rp", bufs=3) as rp,
            tc.tile_pool(name="probs", bufs=4) as probs,
            tc.tile_pool(name="smallp", bufs=2) as smallp,
            tc.tile_pool(name="ap", bufs=3) as apool,
            tc.tile_pool(name="wosb", bufs=2) as wosb,
            tc.tile_pool(name="outp", bufs=2) as outp,
            # PSUM: qk 2 banks + scores 2x2 banks + pv 2 banks = 8
            tc.tile_pool(name="qkps", bufs=2, space="PSUM") as qkps,
            tc.tile_pool(name="spool", bufs=2, space="PSUM") as spool,
            tc.tile_pool(name="pvp", bufs=1, space="PSUM") as pvp,
        ):
            qcat = constp.tile([128, 2 * T], dt.bfloat16)   # 2 local q heads
            kT = constp.tile([128, T], dt.bfloat16)
            vsb = constp.tile([128, T], dt.bfloat16)        # token-major v
            wqkv_sb = constp.tile([128, HT, 512], dt.bfloat16)
            wo_sb = constp.tile([128, HT, 256], dt.bfloat16)
            cc_sb = constp.tile([128, T], dt.bfloat16)
            ss_sb = constp.tile([128, T], dt.bfloat16)
            msk_sb = constp.tile([128, 4, 1024], dt.bfloat16)
            ones_sb = constp.tile([128, 128], dt.bfloat16)
            id_sb = constp.tile([128, 128], dt.bfloat16)

            def load_x(j, x_sb):
                for hq in range(4):
                    nc.sync.dma_start(
                        x_sb[:, hq * 4:(hq + 1) * 4, :],
                        xT[:, hq * 4:(hq + 1) * 4, j * 512:(j + 1) * 512],
                    )

            # interleave weight/x chunks so the first ht-tile MMs start ASAP
            x_tiles = {}
            x_tiles[0] = xp.tile([128, HT, 512], dt.bfloat16, name="x0", tag="x")
            for lo, hi in ((0, 2), (2, 4), (4, 8), (8, 12), (12, 16)):
                nc.sync.dma_start(wqkv_sb[:, lo:hi, :], wqkv[:, lo:hi, :])
                nc.sync.dma_start(
                    x_tiles[0][:, lo:hi, :], xT[:, lo:hi, 0:512],
                )

            # remaining constants after the first x tile is on its way; rope
            # tables first (needed by tile 0's rope), output-side consts last
            nc.sync.dma_start(cc_sb[:, 0:512], cc[:, 0:512])
            nc.sync.dma_start(ss_sb[:, 0:512], ss[:, 0:512])
            nc.sync.dma_start(id_sb[:], ident[:])
            nc.sync.dma_start(cc_sb[:, 512:T], cc[:, 512:T])
            nc.sync.dma_start(ss_sb[:, 512:T], ss[:, 512:T])
            nc.sync.dma_start(msk_sb[:], msk[:])
            nc.sync.dma_start(ones_sb[:], ones128[:])
            nc.sync.dma_start(wo_sb[:], wo[:])

            def emit_qkv(j, x_sb):
                """QKV + RoPE + v-transpose for t-tile j (512 tokens)."""
                tsl = bass.ts(j, 512)
                for ft in range(4):
                    ps = qkps.tile([128, 512], dt.float32, tag="qk")
                    for ht in range(HT):
                        nc.tensor.matmul(
                            ps[:],
                            lhsT=wqkv_sb[:, ht, ft * 128:(ft + 1) * 128],
                            rhs=x_sb[:, ht, :],
                            start=(ht == 0),
                            stop=(ht == HT - 1),
                        )
                    if ft < 3:
                        # q0, q1, k: RoPE
                        sbq = rp.tile([128, 512], dt.bfloat16, name="sbq")
                        nc.scalar.copy(sbq[:], ps[:])
                        tmp = rp.tile([128, 512], dt.bfloat16, name="tmp")
                        nc.gpsimd.dma_start(tmp[0:64, :], sbq[64:128, :])
                        nc.gpsimd.dma_start(tmp[64:128, :], sbq[0:64, :])
                        qcc = rp.tile([128, 512], dt.bfloat16, name="qcc")
                        nc.vector.tensor_tensor(qcc[:], sbq[:], cc_sb[:, tsl], mult)
                        qss = rp.tile([128, 512], dt.bfloat16, name="qss")
                        nc.vector.tensor_tensor(qss[:], tmp[:], ss_sb[:, tsl], mult)
                        if ft < 2:
                            dst = qcat[:, ft * T + j * 512: ft * T + (j + 1) * 512]
                        else:
                            dst = kT[:, tsl]
                        nc.vector.tensor_tensor(dst, qcc[:], qss[:], add)
                    else:
                        # v: feature-major -> PE transpose -> token-major
                        vf = rp.tile([128, 512], dt.bfloat16, name="vf")
                        nc.scalar.copy(vf[:], ps[:])
                        pt = qkps.tile(
                            [128, 512], dt.bfloat16, tag="qk", name="pt",
                            padded_shape=[128, 1024],
                        )
                        for st in range(4):
                            nc.tensor.transpose(
                                pt[:, st * 128:(st + 1) * 128],
                                vf[:, st * 128:(st + 1) * 128],
                                id_sb[:],
                            )
                        nc.vector.tensor_copy(vsb[:, tsl], pt[:])

            def emit_attn(b, qt):
                """Causal attention chunk for 512 q tokens; both local heads."""
                kts = 4 * qt + 4
                pv = pvp.tile([128, 1024], dt.float32, tag="pv")
                acc = smallp.tile([128, 1024], dt.bfloat16, name="acc", tag="acc")
                for kt in range(kts):
                    r = kt - 4 * qt
                    ps_s = spool.tile([128, 1024], dt.float32, tag="sc")
                    for hl in range(2):
                        nc.tensor.matmul(
                            ps_s[:, hl * 512:(hl + 1) * 512],
                            lhsT=kT[:, b * S + kt * 128: b * S + (kt + 1) * 128],
                            rhs=qcat[:, hl * T + b * S + qt * 512:
                                     hl * T + b * S + (qt + 1) * 512],
                            start=True,
                            stop=True,
                        )
                    prob = probs.tile([128, 1024], dt.bfloat16, tag="prob")
                    if r >= 0:
                        stg = probs.tile([128, 1024], dt.bfloat16, name="stg", tag="stg")
                        nc.scalar.activation(stg[:], ps_s[:], Exp, scale=SCALE)
                        nc.vector.tensor_tensor(prob[:], stg[:], msk_sb[:, r, :], mult)
                    else:
                        nc.scalar.activation(prob[:], ps_s[:], Exp, scale=SCALE)
                    for hl in range(2):
                        nc.tensor.matmul(
                            pv[:, hl * 512:(hl + 1) * 512],
                            lhsT=vsb[:, (b * HT + kt) * 128: (b * HT + kt + 1) * 128],
                            rhs=prob[:, hl * 512:(hl + 1) * 512],
                            start=(kt == 0), stop=(kt == kts - 1),
                        )
                    if kt == 0:
                        nc.vector.tensor_copy(acc[:], prob[:])
                    else:
                        nc.vector.tensor_add(acc[:], acc[:], prob[:])
                # partition-reduce + broadcast denominators in one matmul per head
                ps_den = spool.tile([128, 1024], dt.float32, tag="sc", name="ps_den")
                for hl in range(2):
                    nc.tensor.matmul(
                        ps_den[:, hl * 512:(hl + 1) * 512],
                        lhsT=ones_sb[:], rhs=acc[:, hl * 512:(hl + 1) * 512],
                        start=True, stop=True,
                    )
                den_sb = smallp.tile([128, 1024], dt.float32, name="den", tag="den")
                nc.vector.reciprocal_approx_fast(den_sb[:], ps_den[:])
                attn_chunk = dram.tile(
                    [256, 512], dt.bfloat16, name=f"attnc{b}_{qt}", tag=f"ac{b}{qt}"
                )
                for hl in range(2):
                    attn_sb = apool.tile([128, 512], dt.bfloat16, tag="asb")
                    nc.vector.tensor_tensor(
                        attn_sb[:], pv[:, hl * 512:(hl + 1) * 512],
                        den_sb[:, hl * 512:(hl + 1) * 512], mult,
                    )
                    nc.sync.dma_start(attn_chunk[hl * 128:(hl + 1) * 128, :], attn_sb[:])
                ag_out = dram.tile(
                    [HT, 128, 512], dt.bfloat16,
                    addr_space="Shared", name=f"agout{b}_{qt}", tag=f"ag{b}{qt}",
                )
                nc.gpsimd.collective_compute(
                    "AllGather", mybir.AluOpType.bypass, replica_groups=RG8,
                    ins=[attn_chunk.opt()], outs=[ag_out.opt()],
                )
                return ag_out

            def load_asb(ag_out):
                asb = wosb.tile([128, HT, 512], dt.bfloat16, tag="asb")
                for g in range(4):
                    nc.sync.dma_start(
                        asb[:, g * 4:(g + 1) * 4, :],
                        ag_out[g * 4:(g + 1) * 4, :, :].transpose([1, 0, 2]),
                    )
                return asb

            def emit_wo(chunks):
                """W_o for 1-2 chunks of 512 tokens; multi-chunk shares each
                128-column weight load across the chunks' matmuls. Deferred so
                the PE stream never waits on a fresh gather."""
                asbs = [(b, qt, load_asb(ag)) for b, qt, ag in chunks]
                for ct in range(2):
                    pss = [
                        (b, qt, asb,
                         qkps.tile([128, 512], dt.float32, tag="qk", name="ps_o"))
                        for b, qt, asb in asbs
                    ]
                    for dtt in range(HT):
                        for b, qt, asb, ps_o in pss:
                            nc.tensor.matmul(
                                ps_o[:],
                                lhsT=wo_sb[:, dtt, ct * 128:(ct + 1) * 128],
                                rhs=asb[:, dtt, :],
                                start=(dtt == 0), stop=(dtt == HT - 1),
                            )
                    for b, qt, asb, ps_o in pss:
                        o_sb = outp.tile([128, 512], dt.float32, tag="osb")
                        nc.scalar.copy(o_sb[:], ps_o[:])
                        nc.scalar.dma_start(
                            outT[ct * 128:(ct + 1) * 128,
                                 b * S + qt * 512: b * S + (qt + 1) * 512],
                            o_sb[:],
                        )

            pending_wo = []
            for j in range(8):
                b, qt = j // 4, j % 4
                if j + 1 < 8:
                    x_tiles[j + 1] = xp.tile(
                        [128, HT, 512], dt.bfloat16, name=f"x{j + 1}", tag="x"
                    )
                    load_x(j + 1, x_tiles[j + 1])
                emit_qkv(j, x_tiles[j])
                ag = emit_attn(b, qt)
                pending_wo.append((b, qt, ag))
                if len(pending_wo) > 4:
                    emit_wo([pending_wo.pop(0)])
            # tail: remaining chunks in weight-sharing pairs; the last gathers
            # overlap the earlier pairs' matmuls
            emit_wo(pending_wo[0:2])
            emit_wo(pending_wo[2:4])
            emit_wo(pending_wo[4:5])
    nc.compile()
    return nc


# host-side input prep ------------------------------------------------------

_PERM = np.concatenate([np.arange(0, HD, 2), np.arange(1, HD, 2)])


def _rope_tables():
    freq = 1.0 / (10000.0 ** (np.arange(0, HD, 2, dtype=np.float64) / HD))
    pos = np.arange(S, dtype=np.float64)
    ang = np.outer(pos, freq)                       # [S, 64]
    cos = np.cos(ang).T.astype(np.float32)          # [64, S]
    sin = np.sin(ang).T.astype(np.float32)
    cc1 = np.concatenate([cos, cos], 0)             # [128, S]
    ss1 = np.concatenate([-sin, sin], 0)            # [128, S]
    return (np.tile(cc1, (1, B)).astype(BF16), np.tile(ss1, (1, B)).astype(BF16))


def _prep_inputs(x, W_qkv, W_o):
    x = np.asarray(x, dtype=np.float32)
    W_qkv = np.asarray(W_qkv, dtype=np.float32)
    W_o = np.asarray(W_o, dtype=np.float32)

    xx = np.concatenate([x[0], x[1]], axis=0)       # [4096, 2048]
    xTd = np.ascontiguousarray(
        xx.T.reshape(HT, 128, T).transpose(1, 0, 2)
    ).astype(BF16)                                   # [128, HT, 4096]

    cc, ss = _rope_tables()

    mask = np.zeros((128, 4, 1024), dtype=np.float32)
    ii = np.arange(128)[:, None]
    jj = np.arange(512)[None, :]
    for r in range(4):
        m = (jj >= ii + 128 * r)
        mask[:, r, 0:512] = m
        mask[:, r, 512:1024] = m
    mask = mask.astype(BF16)

    ones128 = np.ones((128, 128), dtype=np.float32).astype(BF16)
    ident = np.eye(128, dtype=np.float32).astype(BF16)

    in_maps = []
    for c in range(8):
        kh = c // 2
        qr = W_qkv[256 * c: 256 * (c + 1)]           # rows of q heads 2c,2c+1
        qr = qr.reshape(2, HD, HIDDEN)[:, _PERM, :].reshape(256, HIDDEN)
        kr = W_qkv[HIDDEN + 128 * kh: HIDDEN + 128 * (kh + 1)][_PERM, :]
        vr = W_qkv[HIDDEN + 512 + 128 * kh: HIDDEN + 512 + 128 * (kh + 1)]
        wqkvT = np.ascontiguousarray(
            np.concatenate([qr, kr, vr], 0).T.reshape(HT, 128, 512).transpose(1, 0, 2)
        ).astype(BF16)                               # [128, HT, 512]
        woT = np.ascontiguousarray(
            W_o[256 * c: 256 * (c + 1)].T.reshape(HT, 128, 256).transpose(1, 0, 2)
        ).astype(BF16)
        in_maps.append({
            "xT": xTd, "wqkv": wqkvT, "wo": woT,
            "cc": cc, "ss": ss, "msk": mask, "ones128": ones128, "ident": ident,
        })
    return in_maps


def kernel(x, W_qkv, W_o):
    global _COMPILED
    if _COMPILED is None:
        _COMPILED = _build()
    nc = _COMPILED
    in_maps = _prep_inputs(x, W_qkv, W_o)
    res = run_bass_kernel_spmd(nc, in_maps, list(range(8)))
    out = np.empty((B, S, HIDDEN), dtype=np.float32)
    for c in range(8):
        oT = res.results[c]["outT"]                  # [256, 4096]
        out[:, :, 256 * c: 256 * (c + 1)] = oT.reshape(256, B, S).transpose(1, 2, 0)
    return out


# revision 11
# speedup vs baseline: 1.0921x; 1.0212x over previous
"""Trainium2 Bass kernel for nn_Attention_4183298146960.

GQA causal attention layer: B=2, S=2048, HIDDEN=2048, 16 q heads / 4 kv heads,
head_dim=128, RoPE (interleaved pairs), causal softmax, output projection.

Sharding (8 cores, SPMD-uniform program):
  core c owns q heads {2c, 2c+1} and kv head c//2, for BOTH batches.

Pipeline (v2): fully interleaved per 512-token tile. For t-tile j
(b=j//4, qt=j%4): QKV projection (q0,q1,k,v as four N=512 column blocks of
one packed weight), RoPE on q/k, PE-transpose of v to token-major, then
immediately the causal attention chunk (b,qt) over k-tiles 0..4qt+3, its
AllGather, and (deferred by 2 chunks) the W_o matmul for an earlier chunk.
This keeps ScalarE (exp) and DVE (mask/denominator) busy under the PE-bound
QKV/W_o matmul stream instead of serializing phases.

Layouts on device (partition dim first):
  feature-major qT/kT [head_dim, tokens] for scores; token-major v
  [tokens, head_dim] for PV; scores computed transposed [k, q], both heads
  side by side in one [128, 1024] PSUM tile so softmax needs one exp /
  mask-mult / denominator-add per k-tile. No max-subtraction (scores are
  O(+-10), exp is fp32-safe); denominator via ones-matmul partition-reduce;
  probabilities stay unnormalized until after PV.
  RoPE head dims are permuted [even | odd] via host-side W row permutation
  so the rotation is a 64-partition swap (SBUF->SBUF DMA) + DVE ops.
"""

import numpy as np
import ml_dtypes

import concourse.bass as bass
import concourse.mybir as mybir
import concourse.tile as tile
from concourse import bacc
from concourse.bass_utils import run_bass_kernel_spmd

BF16 = ml_dtypes.bfloat16

HEADS = 16
KV_HEADS = 4
HIDDEN = 2048
HD = 128
S = 2048
B = 2
T = B * S                      # 4096 token axis (both batches)
HT = HIDDEN // 128             # 16 hidden tiles
SCALE = 1.0 / float(np.sqrt(HD))
RG8 = [[0, 1, 2, 3, 4, 5, 6, 7]]

_COMPILED = None


def _build():
    dt = mybir.dt
    nc = bacc.Bacc("TRN2", target_bir_lowering=False, debug=False, num_devices=8)

    xT = nc.dram_tensor("xT", [128, HT, T], dt.bfloat16, kind="ExternalInput")
    wqkv = nc.dram_tensor("wqkv", [128, HT, 512], dt.bfloat16, kind="ExternalInput")
    wo = nc.dram_tensor("wo", [128, HT, 256], dt.bfloat16, kind="ExternalInput")
    cc = nc.dram_tensor("cc", [128, T], dt.bfloat16, kind="ExternalInput")
    ss = nc.dram_tensor("ss", [128, T], dt.bfloat16, kind="ExternalInput")
    msk = nc.dram_tensor("msk", [128, 4, 1024], dt.bfloat16, kind="ExternalInput")
    ones128 = nc.dram_tensor("ones128", [128, 128], dt.bfloat16, kind="ExternalInput")
    ident = nc.dram_tensor("ident", [128, 128], dt.bfloat16, kind="ExternalInput")
    outT = nc.dram_tensor("outT", [256, T], dt.float32, kind="ExternalOutput")

    mult = mybir.AluOpType.mult
    add = mybir.AluOpType.add
    Exp = mybir.ActivationFunctionType.Exp

    with tile.TileContext(nc) as tc:
        with (
            tc.tile_pool(name="const", bufs=1) as constp,
            tc.tile_pool(name="dram", bufs=1, space="DRAM") as dram,
            tc.tile_pool(name="xp", bufs=2) as xp,
            tc.tile_pool(name="rp", bufs=3) as rp,
            tc.tile_pool(name="probs", bufs=4) as probs,
            tc.tile_pool(name="smallp", bufs=2) as smallp,
            tc.tile_pool(name="ap", bufs=3) as apool,
            tc.tile_pool(name="wosb", bufs=3) as wosb,
            tc.tile_pool(name="outp", bufs=2) as outp,
            # PSUM: qk 2 banks + scores 2x2 banks + pv 2 banks = 8
            tc.tile_pool(name="qkps", bufs=2, space="PSUM") as qkps,
            tc.tile_pool(name="spool", bufs=2, space="PSUM") as spool,
            tc.tile_pool(name="pvp", bufs=1, space="PSUM") as pvp,
        ):
            qcat = constp.tile([128, 2 * T], dt.bfloat16)   # 2 local q heads
            kT = constp.tile([128, T], dt.bfloat16)
            vsb = constp.tile([128, T], dt.bfloat16)        # token-major v
            wqkv_sb = constp.tile([128, HT, 512], dt.bfloat16)
            wo_sb = constp.tile([128, HT, 256], dt.bfloat16)
            cc_sb = constp.tile([128, T], dt.bfloat16)
            ss_sb = constp.tile([128, T], dt.bfloat16)
            msk_sb = constp.tile([128, 4, 1024], dt.bfloat16)
            ones_sb = constp.tile([128, 128], dt.bfloat16)
            id_sb = constp.tile([128, 128], dt.bfloat16)

            def load_x(j, x_sb):
                for hq in range(4):
                    nc.sync.dma_start(
                        x_sb[:, hq * 4:(hq + 1) * 4, :],
                        xT[:, hq * 4:(hq + 1) * 4, j * 512:(j + 1) * 512],
                    )

            # interleave weight/x chunks so the first ht-tile MMs start ASAP
            x_tiles = {}
            x_tiles[0] = xp.tile([128, HT, 512], dt.bfloat16, name="x0", tag="x")
            for lo, hi in ((0, 2), (2, 4), (4, 8), (8, 12), (12, 16)):
                nc.sync.dma_start(wqkv_sb[:, lo:hi, :], wqkv[:, lo:hi, :])
                nc.sync.dma_start(
                    x_tiles[0][:, lo:hi, :], xT[:, lo:hi, 0:512],
                )

            # remaining constants after the first x tile is on its way; rope
            # tables first (needed by tile 0's rope), output-side consts last
            nc.sync.dma_start(cc_sb[:, 0:512], cc[:, 0:512])
            nc.sync.dma_start(ss_sb[:, 0:512], ss[:, 0:512])
            nc.sync.dma_start(id_sb[:], ident[:])
            nc.sync.dma_start(cc_sb[:, 512:T], cc[:, 512:T])
            nc.sync.dma_start(ss_sb[:, 512:T], ss[:, 512:T])
            nc.sync.dma_start(msk_sb[:], msk[:])
            nc.sync.dma_start(ones_sb[:], ones128[:])
            nc.sync.dma_start(wo_sb[:], wo[:])

            def emit_qkv(j, x_sb):
                """QKV + RoPE + v-transpose for t-tile j (512 tokens)."""
                tsl = bass.ts(j, 512)
                for ft in range(4):
                    ps = qkps.tile([128, 512], dt.float32, tag="qk")
                    for ht in range(HT):
                        nc.tensor.matmul(
                            ps[:],
                            lhsT=wqkv_sb[:, ht, ft * 128:(ft + 1) * 128],
                            rhs=x_sb[:, ht, :],
                            start=(ht == 0),
                            stop=(ht == HT - 1),
                        )
                    if ft < 3:
                        # q0, q1, k: RoPE
                        sbq = rp.tile([128, 512], dt.bfloat16, name="sbq")
                        nc.scalar.copy(sbq[:], ps[:])
                        tmp = rp.tile([128, 512], dt.bfloat16, name="tmp")
                        nc.gpsimd.dma_start(tmp[0:64, :], sbq[64:128, :])
                        nc.gpsimd.dma_start(tmp[64:128, :], sbq[0:64, :])
                        qcc = rp.tile([128, 512], dt.bfloat16, name="qcc")
                        nc.vector.tensor_tensor(qcc[:], sbq[:], cc_sb[:, tsl], mult)
                        qss = rp.tile([128, 512], dt.bfloat16, name="qss")
                        nc.vector.tensor_tensor(qss[:], tmp[:], ss_sb[:, tsl], mult)
                        if ft < 2:
                            dst = qcat[:, ft * T + j * 512: ft * T + (j + 1) * 512]
                        else:
                            dst = kT[:, tsl]
                        nc.vector.tensor_tensor(dst, qcc[:], qss[:], add)
                    else:
                        # v: feature-major -> PE transpose -> token-major
                        vf = rp.tile([128, 512], dt.bfloat16, name="vf")
                        nc.scalar.copy(vf[:], ps[:])
                        pt = qkps.tile(
                            [128, 512], dt.bfloat16, tag="qk", name="pt",
                            padded_shape=[128, 1024],
                        )
                        for st in range(4):
                            nc.tensor.transpose(
                                pt[:, st * 128:(st + 1) * 128],
                                vf[:, st * 128:(st + 1) * 128],
                                id_sb[:],
                            )
                        nc.vector.tensor_copy(vsb[:, tsl], pt[:])

            def emit_attn(b, qt):
                """Causal attention chunk for 512 q tokens; both local heads."""
                kts = 4 * qt + 4
                pv = pvp.tile([128, 1024], dt.float32, tag="pv")
                acc = smallp.tile([128, 1024], dt.bfloat16, name="acc", tag="acc")
                for kt in range(kts):
                    r = kt - 4 * qt
                    ps_s = spool.tile([128, 1024], dt.float32, tag="sc")
                    for hl in range(2):
                        nc.tensor.matmul(
                            ps_s[:, hl * 512:(hl + 1) * 512],
                            lhsT=kT[:, b * S + kt * 128: b * S + (kt + 1) * 128],
                            rhs=qcat[:, hl * T + b * S + qt * 512:
                                     hl * T + b * S + (qt + 1) * 512],
                            start=True,
                            stop=True,
                        )
                    prob = probs.tile([128, 1024], dt.bfloat16, tag="prob")
                    nc.scalar.activation(prob[:], ps_s[:], Exp, scale=SCALE)
                    if r >= 0:
                        nc.vector.tensor_tensor(prob[:], prob[:], msk_sb[:, r, :], mult)
                    for hl in range(2):
                        nc.tensor.matmul(
                            pv[:, hl * 512:(hl + 1) * 512],
                            lhsT=vsb[:, (b * HT + kt) * 128: (b * HT + kt + 1) * 128],
                            rhs=prob[:, hl * 512:(hl + 1) * 512],
                            start=(kt == 0), stop=(kt == kts - 1),
                        )
                    if kt == 0:
                        nc.vector.tensor_copy(acc[:], prob[:])
                    else:
                        nc.vector.tensor_add(acc[:], acc[:], prob[:])
                # partition-reduce + broadcast denominators in one matmul per head
                ps_den = spool.tile([128, 1024], dt.float32, tag="sc", name="ps_den")
                for hl in range(2):
                    nc.tensor.matmul(
                        ps_den[:, hl * 512:(hl + 1) * 512],
                        lhsT=ones_sb[:], rhs=acc[:, hl * 512:(hl + 1) * 512],
                        start=True, stop=True,
                    )
                den_sb = smallp.tile([128, 1024], dt.float32, name="den", tag="den")
                nc.vector.reciprocal_approx_fast(den_sb[:], ps_den[:])
                attn_chunk = dram.tile(
                    [256, 512], dt.bfloat16, name=f"attnc{b}_{qt}", tag=f"ac{b}{qt}"
                )
                for hl in range(2):
                    attn_sb = apool.tile([128, 512], dt.bfloat16, tag="asb")
                    nc.vector.tensor_tensor(
                        attn_sb[:], pv[:, hl * 512:(hl + 1) * 512],
                        den_sb[:, hl * 512:(hl + 1) * 512], mult,
                    )
                    nc.sync.dma_start(attn_chunk[hl * 128:(hl + 1) * 128, :], attn_sb[:])
                ag_out = dram.tile(
                    [HT, 128, 512], dt.bfloat16,
                    addr_space="Shared", name=f"agout{b}_{qt}", tag=f"ag{b}{qt}",
                )
                nc.gpsimd.collective_compute(
                    "AllGather", mybir.AluOpType.bypass, replica_groups=RG8,
                    ins=[attn_chunk.opt()], outs=[ag_out.opt()],
                )
                return ag_out

            def load_asb(ag_out):
                # NB: must be emitted BEFORE the next collective so the
                # framework's shared collective-completion counter makes this
                # wait only on the producing AllGather, not a later one.
                asb = wosb.tile([128, HT, 512], dt.bfloat16, tag="asb")
                for g in range(4):
                    nc.sync.dma_start(
                        asb[:, g * 4:(g + 1) * 4, :],
                        ag_out[g * 4:(g + 1) * 4, :, :].transpose([1, 0, 2]),
                    )
                return asb

            def emit_wo(chunks):
                """W_o matmuls for 1-2 already-loaded chunks; multi-chunk
                shares each 128-column weight load across chunks."""
                for ct in range(2):
                    pss = [
                        (b, qt, asb,
                         qkps.tile([128, 512], dt.float32, tag="qk", name="ps_o"))
                        for b, qt, asb in chunks
                    ]
                    for dtt in range(HT):
                        for b, qt, asb, ps_o in pss:
                            nc.tensor.matmul(
                                ps_o[:],
                                lhsT=wo_sb[:, dtt, ct * 128:(ct + 1) * 128],
                                rhs=asb[:, dtt, :],
                                start=(dtt == 0), stop=(dtt == HT - 1),
                            )
                    for b, qt, asb, ps_o in pss:
                        o_sb = outp.tile([128, 512], dt.float32, tag="osb")
                        nc.scalar.copy(o_sb[:], ps_o[:])
                        nc.scalar.dma_start(
                            outT[ct * 128:(ct + 1) * 128,
                                 b * S + qt * 512: b * S + (qt + 1) * 512],
                            o_sb[:],
                        )

            pending_ag = []    # chunks gathered but asb not yet loaded
            loaded = []        # chunks with asb in SBUF, wo not yet emitted
            for j in range(8):
                b, qt = j // 4, j % 4
                if j + 1 < 8:
                    x_tiles[j + 1] = xp.tile(
                        [128, HT, 512], dt.bfloat16, name=f"x{j + 1}", tag="x"
                    )
                    load_x(j + 1, x_tiles[j + 1])
                emit_qkv(j, x_tiles[j])
                # load chunk j-1's gather result before emitting AllGather j
                if pending_ag:
                    bb, qq, ag = pending_ag.pop(0)
                    loaded.append((bb, qq, load_asb(ag)))
                if len(loaded) > 2:
                    emit_wo([loaded.pop(0)])
                ag = emit_attn(b, qt)
                pending_ag.append((b, qt, ag))
            # tail: chunks 5,6 have asb loaded (or loading); 7 waits its gather
            emit_wo(loaded[0:2])
            bb, qq, ag = pending_ag.pop(0)
            emit_wo([(bb, qq, load_asb(ag))])
    nc.compile()
    return nc


# host-side input prep ------------------------------------------------------

_PERM = np.concatenate([np.arange(0, HD, 2), np.arange(1, HD, 2)])


def _rope_tables():
    freq = 1.0 / (10000.0 ** (np.arange(0, HD, 2, dtype=np.float64) / HD))
    pos = np.arange(S, dtype=np.float64)
    ang = np.outer(pos, freq)                       # [S, 64]
    cos = np.cos(ang).T.astype(np.float32)          # [64, S]
    sin = np.sin(ang).T.astype(np.float32)
    cc1 = np.concatenate([cos, cos], 0)             # [128, S]
    ss1 = np.concatenate([-sin, sin], 0)            # [128, S]
    return (np.tile(cc1, (1, B)).astype(BF16), np.tile(ss1, (1, B)).astype(BF16))


def _prep_inputs(x, W_qkv, W_o):
    x = np.asarray(x, dtype=np.float32)
    W_qkv = np.asarray(W_qkv, dtype=np.float32)
    W_o = np.asarray(W_o, dtype=np.float32)

    xx = np.concatenate([x[0], x[1]], axis=0)       # [4096, 2048]
    xTd = np.ascontiguousarray(
        xx.T.reshape(HT, 128, T).transpose(1, 0, 2)
    ).astype(BF16)                                   # [128, HT, 4096]

    cc, ss = _rope_tables()

    mask = np.zeros((128, 4, 1024), dtype=np.float32)
    ii = np.arange(128)[:, None]
    jj = np.arange(512)[None, :]
    for r in range(4):
        m = (jj >= ii + 128 * r)
        mask[:, r, 0:512] = m
        mask[:, r, 512:1024] = m
    mask = mask.astype(BF16)

    ones128 = np.ones((128, 128), dtype=np.float32).astype(BF16)
    ident = np.eye(128, dtype=np.float32).astype(BF16)

    in_maps = []
    for c in range(8):
        kh = c // 2
        qr = W_qkv[256 * c: 256 * (c + 1)]           # rows of q heads 2c,2c+1
        qr = qr.reshape(2, HD, HIDDEN)[:, _PERM, :].reshape(256, HIDDEN)
        kr = W_qkv[HIDDEN + 128 * kh: HIDDEN + 128 * (kh + 1)][_PERM, :]
        vr = W_qkv[HIDDEN + 512 + 128 * kh: HIDDEN + 512 + 128 * (kh + 1)]
        wqkvT = np.ascontiguousarray(
            np.concatenate([qr, kr, vr], 0).T.reshape(HT, 128, 512).transpose(1, 0, 2)
        ).astype(BF16)                               # [128, HT, 512]
        woT = np.ascontiguousarray(
            W_o[256 * c: 256 * (c + 1)].T.reshape(HT, 128, 256).transpose(1, 0, 2)
        ).astype(BF16)
        in_maps.append({
            "xT": xTd, "wqkv": wqkvT, "wo": woT,
            "cc": cc, "ss": ss, "msk": mask, "ones128": ones128, "ident": ident,
        })
    return in_maps


def kernel(x, W_qkv, W_o):
    global _COMPILED
    if _COMPILED is None:
        _COMPILED = _build()
    nc = _COMPILED
    in_maps = _prep_inputs(x, W_qkv, W_o)
    res = run_bass_kernel_spmd(nc, in_maps, list(range(8)))
    out = np.empty((B, S, HIDDEN), dtype=np.float32)
    for c in range(8):
        oT = res.results[c]["outT"]                  # [256, 4096]
        out[:, :, 256 * c: 256 * (c + 1)] = oT.reshape(256, B, S).transpose(1, 2, 0)
    return out


# revision 13
# speedup vs baseline: 1.1607x; 1.0628x over previous
"""Trainium2 Bass kernel for nn_Attention_4183298146960 — v6 sharding.

GQA causal attention layer: B=2, S=2048, HIDDEN=2048, 16 q heads / 4 kv heads,
head_dim=128, RoPE (interleaved pairs), causal softmax, output projection.

Sharding v6 (8 cores, SPMD-uniform program, per-core inputs differ):
  core c owns batch c//4 and kv-head g=c%4 with its 4 q heads {4g..4g+3}.
  Eliminates the duplicated K/V projections of head-sharding (each (batch,
  kv-head) is computed exactly once), halves the x DMA per core, and the
  output-projection gather becomes an AllGather within each 4-core batch
  group (6 MB received per core instead of 14 MB).

Pipeline: per 512-token tile j (= qt chunk of my batch): QKV (q0..q3,k,v as
six N=512 column blocks of one packed weight), RoPE on q/k, PE-transpose of
v to token-major, then the causal attention chunk over k-tiles 0..4qt+3 in
two head-pair passes, its AllGather, the previous chunk's gather-readback
(emitted before the new AllGather so the framework's shared collective
counter waits only on the producing gather), and a deferred chunk's W_o.

Layouts as before: feature-major qT/kT for scores, token-major v for PV,
scores [k, q] with a head-pair side by side in one [128, 1024] PSUM tile;
unnormalized exp, ones-matmul denominator, late division.
"""

import numpy as np
import ml_dtypes

import concourse.bass as bass
import concourse.mybir as mybir
import concourse.tile as tile
from concourse import bacc
from concourse.bass_utils import run_bass_kernel_spmd

BF16 = ml_dtypes.bfloat16

HEADS = 16
KV_HEADS = 4
HIDDEN = 2048
HD = 128
S = 2048
B = 2
HT = HIDDEN // 128             # 16 hidden tiles
NQ = 4                         # local q heads
SCALE = 1.0 / float(np.sqrt(HD))
RG4 = [[0, 1, 2, 3], [4, 5, 6, 7]]

_COMPILED = None


def _build():
    dt = mybir.dt
    nc = bacc.Bacc("TRN2", target_bir_lowering=False, debug=False, num_devices=8)

    xT = nc.dram_tensor("xT", [128, HT, S], dt.bfloat16, kind="ExternalInput")
    wqkv = nc.dram_tensor("wqkv", [128, HT, 768], dt.bfloat16, kind="ExternalInput")
    wo = nc.dram_tensor("wo", [128, HT, 512], dt.bfloat16, kind="ExternalInput")
    cc = nc.dram_tensor("cc", [128, S], dt.bfloat16, kind="ExternalInput")
    ss = nc.dram_tensor("ss", [128, S], dt.bfloat16, kind="ExternalInput")
    msk = nc.dram_tensor("msk", [128, 4, 1024], dt.bfloat16, kind="ExternalInput")
    ones128 = nc.dram_tensor("ones128", [128, 128], dt.bfloat16, kind="ExternalInput")
    ident = nc.dram_tensor("ident", [128, 128], dt.bfloat16, kind="ExternalInput")
    outT = nc.dram_tensor("outT", [512, S], dt.float32, kind="ExternalOutput")

    mult = mybir.AluOpType.mult
    add = mybir.AluOpType.add
    Exp = mybir.ActivationFunctionType.Exp

    with tile.TileContext(nc) as tc:
        with (
            tc.tile_pool(name="const", bufs=1) as constp,
            tc.tile_pool(name="dram", bufs=1, space="DRAM") as dram,
            tc.tile_pool(name="xp", bufs=2) as xp,
            tc.tile_pool(name="rp", bufs=3) as rp,
            tc.tile_pool(name="probs", bufs=4) as probs,
            tc.tile_pool(name="smallp", bufs=2) as smallp,
            tc.tile_pool(name="ap", bufs=3) as apool,
            tc.tile_pool(name="wosb", bufs=3) as wosb,
            tc.tile_pool(name="outp", bufs=2) as outp,
            # PSUM: qk 2 banks + scores 2x2 banks + pv 2 banks = 8
            tc.tile_pool(name="qkps", bufs=2, space="PSUM") as qkps,
            tc.tile_pool(name="spool", bufs=2, space="PSUM") as spool,
            tc.tile_pool(name="pvp", bufs=1, space="PSUM") as pvp,
        ):
            qcat = constp.tile([128, NQ * S], dt.bfloat16)   # 4 local q heads
            kT = constp.tile([128, S], dt.bfloat16)
            vsb = constp.tile([128, S], dt.bfloat16)         # token-major v
            wqkv_sb = constp.tile([128, HT, 768], dt.bfloat16)
            wo_sb = constp.tile([128, HT, 512], dt.bfloat16)
            cc_sb = constp.tile([128, S], dt.bfloat16)
            ss_sb = constp.tile([128, S], dt.bfloat16)
            msk_sb = constp.tile([128, 4, 1024], dt.bfloat16)
            ones_sb = constp.tile([128, 128], dt.bfloat16)
            id_sb = constp.tile([128, 128], dt.bfloat16)

            def load_x(j, x_sb):
                for hq in range(4):
                    nc.sync.dma_start(
                        x_sb[:, hq * 4:(hq + 1) * 4, :],
                        xT[:, hq * 4:(hq + 1) * 4, j * 512:(j + 1) * 512],
                    )

            # interleave weight/x chunks so the first ht-tile MMs start ASAP
            x_tiles = {}
            x_tiles[0] = xp.tile([128, HT, 512], dt.bfloat16, name="x0", tag="x")
            for lo, hi in ((0, 2), (2, 4), (4, 8), (8, 12), (12, 16)):
                nc.sync.dma_start(wqkv_sb[:, lo:hi, :], wqkv[:, lo:hi, :])
                nc.sync.dma_start(
                    x_tiles[0][:, lo:hi, :], xT[:, lo:hi, 0:512],
                )

            nc.sync.dma_start(cc_sb[:, 0:512], cc[:, 0:512])
            nc.sync.dma_start(ss_sb[:, 0:512], ss[:, 0:512])
            nc.sync.dma_start(id_sb[:], ident[:])
            nc.sync.dma_start(cc_sb[:, 512:S], cc[:, 512:S])
            nc.sync.dma_start(ss_sb[:, 512:S], ss[:, 512:S])
            nc.sync.dma_start(msk_sb[:], msk[:])
            nc.sync.dma_start(ones_sb[:], ones128[:])
            nc.sync.dma_start(wo_sb[:], wo[:])

            def emit_qkv(j, x_sb):
                """QKV + RoPE + v-transpose for t-tile j (512 tokens)."""
                tsl = bass.ts(j, 512)
                for ft in range(6):
                    ps = qkps.tile([128, 512], dt.float32, tag="qk")
                    for ht in range(HT):
                        nc.tensor.matmul(
                            ps[:],
                            lhsT=wqkv_sb[:, ht, ft * 128:(ft + 1) * 128],
                            rhs=x_sb[:, ht, :],
                            start=(ht == 0),
                            stop=(ht == HT - 1),
                        )
                    if ft < 5:
                        # q0..q3, k: RoPE
                        sbq = rp.tile([128, 512], dt.bfloat16, name="sbq")
                        nc.scalar.copy(sbq[:], ps[:])
                        tmp = rp.tile([128, 512], dt.bfloat16, name="tmp")
                        nc.gpsimd.dma_start(tmp[0:64, :], sbq[64:128, :])
                        nc.gpsimd.dma_start(tmp[64:128, :], sbq[0:64, :])
                        qcc = rp.tile([128, 512], dt.bfloat16, name="qcc")
                        nc.vector.tensor_tensor(qcc[:], sbq[:], cc_sb[:, tsl], mult)
                        qss = rp.tile([128, 512], dt.bfloat16, name="qss")
                        nc.vector.tensor_tensor(qss[:], tmp[:], ss_sb[:, tsl], mult)
                        if ft < 4:
                            dst = qcat[:, ft * S + j * 512: ft * S + (j + 1) * 512]
                        else:
                            dst = kT[:, tsl]
                        nc.vector.tensor_tensor(dst, qcc[:], qss[:], add)
                    else:
                        # v: feature-major -> PE transpose -> token-major
                        vf = rp.tile([128, 512], dt.bfloat16, name="vf")
                        nc.scalar.copy(vf[:], ps[:])
                        pt = qkps.tile(
                            [128, 512], dt.bfloat16, tag="qk", name="pt",
                            padded_shape=[128, 1024],
                        )
                        for st in range(4):
                            nc.tensor.transpose(
                                pt[:, st * 128:(st + 1) * 128],
                                vf[:, st * 128:(st + 1) * 128],
                                id_sb[:],
                            )
                        nc.vector.tensor_copy(vsb[:, tsl], pt[:])

            def emit_attn(qt):
                """Causal attention chunk for 512 q tokens; 4 local heads in
                two head-pair passes."""
                kts = 4 * qt + 4
                attn_chunk = dram.tile(
                    [512, 512], dt.bfloat16, name=f"attnc{qt}", tag=f"ac{qt}"
                )
                for hp in range(2):
                    pv = pvp.tile([128, 1024], dt.float32, tag="pv")
                    acc = smallp.tile([128, 1024], dt.bfloat16, name="acc", tag="acc")
                    for kt in range(kts):
                        r = kt - 4 * qt
                        ps_s = spool.tile([128, 1024], dt.float32, tag="sc")
                        for hl in range(2):
                            h = 2 * hp + hl
                            nc.tensor.matmul(
                                ps_s[:, hl * 512:(hl + 1) * 512],
                                lhsT=kT[:, kt * 128:(kt + 1) * 128],
                                rhs=qcat[:, h * S + qt * 512: h * S + (qt + 1) * 512],
                                start=True,
                                stop=True,
                            )
                        prob = probs.tile([128, 1024], dt.bfloat16, tag="prob")
                        nc.scalar.activation(prob[:], ps_s[:], Exp, scale=SCALE)
                        if r >= 0:
                            nc.vector.tensor_tensor(
                                prob[:], prob[:], msk_sb[:, r, :], mult
                            )
                        for hl in range(2):
                            nc.tensor.matmul(
                                pv[:, hl * 512:(hl + 1) * 512],
                                lhsT=vsb[:, kt * 128:(kt + 1) * 128],
                                rhs=prob[:, hl * 512:(hl + 1) * 512],
                                start=(kt == 0), stop=(kt == kts - 1),
                            )
                        if kt == 0:
                            nc.vector.tensor_copy(acc[:], prob[:])
                        else:
                            nc.vector.tensor_add(acc[:], acc[:], prob[:])
                    ps_den = spool.tile([128, 1024], dt.float32, tag="sc", name="ps_den")
                    for hl in range(2):
                        nc.tensor.matmul(
                            ps_den[:, hl * 512:(hl + 1) * 512],
                            lhsT=ones_sb[:], rhs=acc[:, hl * 512:(hl + 1) * 512],
                            start=True, stop=True,
                        )
                    den_sb = smallp.tile([128, 1024], dt.float32, name="den", tag="den")
                    nc.vector.reciprocal_approx_fast(den_sb[:], ps_den[:])
                    for hl in range(2):
                        h = 2 * hp + hl
                        attn_sb = apool.tile([128, 512], dt.bfloat16, tag="asb")
                        nc.vector.tensor_tensor(
                            attn_sb[:], pv[:, hl * 512:(hl + 1) * 512],
                            den_sb[:, hl * 512:(hl + 1) * 512], mult,
                        )
                        nc.sync.dma_start(
                            attn_chunk[h * 128:(h + 1) * 128, :], attn_sb[:]
                        )
                ag_out = dram.tile(
                    [HT, 128, 512], dt.bfloat16, name=f"agout{qt}", tag=f"ag{qt}",
                )
                nc.gpsimd.collective_compute(
                    "AllGather", mybir.AluOpType.bypass, replica_groups=RG4,
                    ins=[attn_chunk.opt()], outs=[ag_out.opt()],
                )
                return ag_out

            def load_asb(ag_out):
                # NB: must be emitted BEFORE the next collective so the
                # framework's shared collective-completion counter makes this
                # wait only on the producing AllGather, not a later one.
                asb = wosb.tile([128, HT, 512], dt.bfloat16, tag="asb")
                for g in range(4):
                    nc.sync.dma_start(
                        asb[:, g * 4:(g + 1) * 4, :],
                        ag_out[g * 4:(g + 1) * 4, :, :].transpose([1, 0, 2]),
                    )
                return asb

            def emit_wo(qt, asb, cts):
                """W_o matmuls (my 512 output columns x this chunk's tokens)."""
                for ct in cts:
                    ps_o = qkps.tile([128, 512], dt.float32, tag="qk", name="ps_o")
                    for dtt in range(HT):
                        nc.tensor.matmul(
                            ps_o[:],
                            lhsT=wo_sb[:, dtt, ct * 128:(ct + 1) * 128],
                            rhs=asb[:, dtt, :],
                            start=(dtt == 0), stop=(dtt == HT - 1),
                        )
                    o_sb = outp.tile([128, 512], dt.float32, tag="osb")
                    nc.scalar.copy(o_sb[:], ps_o[:])
                    nc.scalar.dma_start(
                        outT[ct * 128:(ct + 1) * 128, qt * 512:(qt + 1) * 512],
                        o_sb[:],
                    )

            pending_ag = []    # chunks gathered, asb not yet loaded
            loaded = []        # chunks with asb in SBUF, wo not yet emitted
            for j in range(4):
                if j + 1 < 4:
                    x_tiles[j + 1] = xp.tile(
                        [128, HT, 512], dt.bfloat16, name=f"x{j + 1}", tag="x"
                    )
                    load_x(j + 1, x_tiles[j + 1])
                emit_qkv(j, x_tiles[j])
                if pending_ag:
                    qq, ag = pending_ag.pop(0)
                    loaded.append((qq, load_asb(ag)))
                if len(loaded) > 1:
                    qq, asb = loaded.pop(0)
                    emit_wo(qq, asb, range(4))
                ag = emit_attn(j)
                pending_ag.append((j, ag))
            # tail: chunk 2's asb is loaded; chunk 3 waits its gather.
            # Split chunk 2's W_o around the final gather so the PE stays busy.
            qq2, asb2 = loaded.pop(0)
            emit_wo(qq2, asb2, range(4))
            qq3, ag3 = pending_ag.pop(0)
            asb3 = load_asb(ag3)
            emit_wo(qq3, asb3, range(4))
    nc.compile()
    return nc


# host-side input prep ------------------------------------------------------

_PERM = np.concatenate([np.arange(0, HD, 2), np.arange(1, HD, 2)])


def _rope_tables():
    freq = 1.0 / (10000.0 ** (np.arange(0, HD, 2, dtype=np.float64) / HD))
    pos = np.arange(S, dtype=np.float64)
    ang = np.outer(pos, freq)                       # [S, 64]
    cos = np.cos(ang).T.astype(np.float32)          # [64, S]
    sin = np.sin(ang).T.astype(np.float32)
    cc1 = np.concatenate([cos, cos], 0)             # [128, S]
    ss1 = np.concatenate([-sin, sin], 0)            # [128, S]
    return cc1.astype(BF16), ss1.astype(BF16)


def _prep_inputs(x, W_qkv, W_o):
    x = np.asarray(x, dtype=np.float32)
    W_qkv = np.asarray(W_qkv, dtype=np.float32)
    W_o = np.asarray(W_o, dtype=np.float32)

    xTd = [
        np.ascontiguousarray(
            x[b].T.reshape(HT, 128, S).transpose(1, 0, 2)
        ).astype(BF16)                               # [128, HT, 2048]
        for b in range(B)
    ]

    cc, ss = _rope_tables()

    mask = np.zeros((128, 4, 1024), dtype=np.float32)
    ii = np.arange(128)[:, None]
    jj = np.arange(512)[None, :]
    for r in range(4):
        m = (jj >= ii + 128 * r)
        mask[:, r, 0:512] = m
        mask[:, r, 512:1024] = m
    mask = mask.astype(BF16)

    ones128 = np.ones((128, 128), dtype=np.float32).astype(BF16)
    ident = np.eye(128, dtype=np.float32).astype(BF16)

    in_maps = []
    for c in range(8):
        b, g = c // 4, c % 4
        qr = W_qkv[512 * g: 512 * (g + 1)]           # rows of q heads 4g..4g+3
        qr = qr.reshape(NQ, HD, HIDDEN)[:, _PERM, :].reshape(512, HIDDEN)
        kr = W_qkv[HIDDEN + 128 * g: HIDDEN + 128 * (g + 1)][_PERM, :]
        vr = W_qkv[HIDDEN + 512 + 128 * g: HIDDEN + 512 + 128 * (g + 1)]
        wqkvT = np.ascontiguousarray(
            np.concatenate([qr, kr, vr], 0).T.reshape(HT, 128, 768).transpose(1, 0, 2)
        ).astype(BF16)                               # [128, HT, 768]
        woT = np.ascontiguousarray(
            W_o[512 * g: 512 * (g + 1)].T.reshape(HT, 128, 512).transpose(1, 0, 2)
        ).astype(BF16)
        in_maps.append({
            "xT": xTd[b], "wqkv": wqkvT, "wo": woT,
            "cc": cc, "ss": ss, "msk": mask, "ones128": ones128, "ident": ident,
        })
    return in_maps


def kernel(x, W_qkv, W_o):
    global _COMPILED
    if _COMPILED is None:
        _COMPILED = _build()
    nc = _COMPILED
    in_maps = _prep_inputs(x, W_qkv, W_o)
    res = run_bass_kernel_spmd(nc, in_maps, list(range(8)))
    out = np.empty((B, S, HIDDEN), dtype=np.float32)
    for c in range(8):
        b, g = c // 4, c % 4
        oT = res.results[c]["outT"]                  # [512, 2048]
        out[b, :, 512 * g: 512 * (g + 1)] = oT.T
    return out
